# revision 10
# baseline (speedup 1.0000x reference)
"""MoE adapter layer kernel for Trainium2 (8 NeuronCores, data-parallel over B).

Reference computation (per sample b):
    pooled = x[b].mean(axis=0)                       # (D,)
    gate   = softmax(pooled @ gate_w.T)              # (E,)
    top2 values/indices, renormalized weights w0,w1
    h_k    = gelu(x[b] @ Wd[ik].T + bd[ik])          # (S, BN)
    out[b] = sum_k w_k * h_k @ Wu[ik].T + sum_k w_k * bu[ik]

Shapes: B=32, S=2048, D=1024, BN=64, E=8, K=2. Inputs fp32.

Strategy: shard B over the 8 cores (4 samples each); replicate the tiny
adapter/gate params. All matmul operands are cast to bf16 on the host
(PE runs 1 cycle/row in bf16 vs 4 in fp32, and HBM traffic halves); the
bf16 rounding of x perturbs the routing logits 2 orders of magnitude
less than the smallest top-2 decision margin, and the end-to-end error
stays ~5e-3 relative vs the 2e-2 gate.

Per core the layer is DMA-bound-ish (~36 MB of HBM traffic) with every
compute engine moderately loaded, so the schedule is built around two
rules: the SP DMA ring carries ONLY the x^T loads (any routing-gated
DMA there stalls the next samples' loads behind a semaphore), and no
engine's program order may make an early sample's work wait on a later
sample's data. Concretely: loads for all 4 samples issue up front
(all of x fits in SBUF in bf16); pooling is per-tile DVE/ACT/GpSimd
reduces that chase the loads, with samples 2-3's reduces interleaved
into the up-matmul loop of samples 0-1; the next sample's gate-logit
matmul is dropped into the middle of the current up loop (right when
its pooling completes); routing needs no softmax (top-2 on logits; the
renormalized pair weight is exactly sigmoid(l1-l2), one ACT op);
expert params are gathered with dynamically-indexed DMAs from
host-prearranged slot-doubled tables on the ACT/GpSimd rings; the
output is produced transposed (d on partitions) so the routed up-bias
is a per-partition scalar fused into the PSUM->SBUF bf16 down-convert,
which alternates ACT/DVE to keep pace with the PE; stores run on the
GpSimd ring. The host un-transposes and upcasts.
"""

import sys

sys.path.insert(0, "/opt/trn_rl_repo")

import numpy as np

import concourse.bass as bass
import concourse.mybir as mybir
import concourse.tile as tile

F32 = mybir.dt.float32
BF16 = mybir.dt.bfloat16
AF = mybir.ActivationFunctionType
ALU = mybir.AluOpType

B, S, D, BN, E = 32, 2048, 1024, 64, 8
NCORES = 8
BPC = B // NCORES  # samples per core
NDC = D // 128     # 8 d-chunks of 128

# engine per (dh, sh) output chunk copy: "s"=ACT, "v"=DVE (gpsimd cannot
# read PSUM). ACT leads so the first chunks drain while DVE finishes the
# next sample's routing.
COPY_ROT = ["s", "s", "v", "v", "s", "v", "s", "v",
            "s", "v", "s", "v", "s", "v", "s", "v"]
# pooling engine per dc chunk: DVE reduce has no fast modes, so ACT
# (activation accum) takes a share to keep DVE off the critical path
POOL_ENG = ["v", "v", "v", "v", "v", "v", "s", "s"]


def _split_multiwait(nc):
    """The pinned walrus encodes at most one sync-wait per instruction;
    hoist extra waits into standalone EventSemaphore instructions."""
    fixn = 0
    for f in nc.m.functions:
        for b in f.blocks:
            if not any(
                i.sync_info is not None
                and i.sync_info.on_wait is not None
                and len(i.sync_info.on_wait) > 1
                for i in b.instructions
            ):
                continue
            out = []
            for inst in b.instructions:
                si = inst.sync_info
                if si is not None and si.on_wait is not None and len(si.on_wait) > 1:
                    waits = list(si.on_wait)
                    for w in waits[:-1]:
                        ev = mybir.InstEventSemaphore(
                            name=f"I-mwfix-{fixn}", engine=inst.engine
                        )
                        ev.sync_info = mybir.SyncInfo(on_wait=[w], on_update=[])
                        out.append(ev)
                        fixn += 1
                    inst.sync_info = mybir.SyncInfo(
                        on_wait=[waits[-1]],
                        on_update=list(si.on_update) if si.on_update else [],
                    )
                out.append(inst)
            b.instructions = out
    return fixn


def build_nc():
    """Build the per-core Bass program (SPMD: same program, different x shard)."""
    nc = bass.Bass()

    # x arrives pre-transposed per sample: (BPC, D, S) bf16 so the down
    # matmul's moving operand (contraction over D -> D on partitions) DMAs
    # naturally.
    xt_in = nc.dram_tensor("xt", [BPC, D, S], BF16, kind="ExternalInput")
    gwt = nc.dram_tensor("gwt", [D, E], F32, kind="ExternalInput")  # gate_w.T/S
    # wdt2[e, p, dc, 64k+c] = down_w[e, c, dc*128+p] for both k slots
    wdt2 = nc.dram_tensor("wdt2", [E, 128, NDC, 128], BF16, kind="ExternalInput")
    # wut2[e, 64k+p, d] = up_w[e, d, p] for both k slots
    wut2 = nc.dram_tensor("wut2", [E, 128, D], BF16, kind="ExternalInput")
    # bcomb[e, p, 0] = down_b[e, p % 64]; bcomb[e, p, 1+dh] = up_b[e, dh*128+p]
    bcomb = nc.dram_tensor("bcomb", [E, 128, 1 + NDC], F32, kind="ExternalInput")
    iota8 = nc.dram_tensor("iota8", [1, E], F32, kind="ExternalInput")
    out_t = nc.dram_tensor("out", [BPC, D, S], BF16, kind="ExternalOutput")
    wts_dram = [nc.dram_tensor(f"wts_scratch_{b}", [1, 2], F32) for b in range(BPC)]

    with tile.TileContext(nc) as tc:
        with (
            tc.tile_pool(name="singles", bufs=1) as singles,
            tc.tile_pool(name="xt", bufs=4 * NDC) as xt_p,
            tc.tile_pool(name="ht", bufs=2) as ht_p,
            tc.tile_pool(name="wg", bufs=2) as wg_p,
            tc.tile_pool(name="osb", bufs=3) as osb_p,
            tc.tile_pool(name="route", bufs=2) as route_p,
            tc.tile_pool(name="hps", bufs=2, space="PSUM") as hps_p,
            tc.tile_pool(name="ops", bufs=2, space="PSUM") as ops_p,
        ):
            gwt_sb = singles.tile([128, NDC, E], F32, tag="gwt")
            nc.sync.dma_start(gwt_sb[:], gwt.rearrange("(dc p) e -> p dc e", p=128))
            iota_sb = singles.tile([1, E], F32, tag="iota")
            nc.sync.dma_start(iota_sb[:], iota8[:])

            state = {b: {} for b in range(BPC)}

            def loads(b):
                # SP ring carries nothing but these
                xt = []
                for dc in range(NDC):
                    xt_sb = xt_p.tile([128, S], BF16, tag="xt", name=f"xt_{b}_{dc}")
                    nc.sync.dma_start(xt_sb[:], xt_in[b, dc * 128:(dc + 1) * 128, :])
                    xt.append(xt_sb)
                state[b]["xt"] = xt
                state[b]["pooled"] = route_p.tile([128, NDC], F32, tag="pooled",
                                                  name=f"pooled_{b}")

            def pool_chunk(b, dc):
                pooled, xt_sb = state[b]["pooled"], state[b]["xt"][dc]
                dst = pooled[:, dc:dc + 1]
                if POOL_ENG[dc] == "v":
                    nc.vector.tensor_reduce(dst, xt_sb[:], mybir.AxisListType.X,
                                            ALU.add)
                else:
                    scr = route_p.tile([128, S], BF16, tag="scr",
                                       name=f"scr_{b}_{dc}")
                    nc.scalar.activation(scr[:], xt_sb[:], AF.Copy, accum_out=dst)

            def logits(b):
                # PE: 8 tiny rank-128 matmuls into an ops-pool corner
                pooled = state[b]["pooled"]
                lps = ops_p.tile([128, 1024], F32, tag="ops", name=f"lps_{b}")
                for dc in range(NDC):
                    nc.tensor.matmul(
                        lps[0:1, 0:E], pooled[:, dc:dc + 1], gwt_sb[:, dc, :],
                        start=(dc == 0), stop=(dc == NDC - 1),
                    )
                state[b]["lps"] = lps

            def route(b):
                st = state[b]
                logits_r = route_p.tile([1, E], F32, tag="logits", name=f"lg_{b}")
                nc.vector.tensor_copy(logits_r[:], st["lps"][0:1, 0:E])

                # top-2 of logits; pair weight = sigmoid(l1-l2) (exactly the
                # renormalized softmax pair, sans the reference's 1e-8 eps)
                m8 = route_p.tile([1, E], F32, tag="m8")
                nc.vector.max(m8[:], logits_r[:])
                ds2 = route_p.tile([1, 2], F32, tag="ds2")
                nc.vector.tensor_sub(ds2[:, 0:1], m8[:, 0:1], m8[:, 1:2])
                nc.vector.tensor_scalar_mul(ds2[:, 1:2], ds2[:, 0:1], -1.0)
                wts = route_p.tile([1, 2], F32, tag="wts")
                nc.scalar.activation(wts[:], ds2[:], AF.Sigmoid)

                idx_i = []
                for k in range(2):
                    eq = route_p.tile([1, E], F32, tag=f"eq{k}")
                    nc.vector.tensor_scalar(eq[:], logits_r[:], m8[:, k:k + 1],
                                            None, ALU.is_equal)
                    # cand = iota*eq + 99*(1-eq): first matching index wins min
                    t1 = route_p.tile([1, E], F32, tag=f"t1_{k}")
                    nc.vector.tensor_mul(t1[:], iota_sb[:], eq[:])
                    t2 = route_p.tile([1, E], F32, tag=f"t2_{k}")
                    nc.vector.tensor_scalar(t2[:], eq[:], -99.0, 99.0, ALU.mult,
                                            ALU.add)
                    cand = route_p.tile([1, E], F32, tag=f"cand{k}")
                    nc.vector.tensor_add(cand[:], t1[:], t2[:])
                    idxf = route_p.tile([1, 1], F32, tag=f"idxf{k}")
                    nc.vector.tensor_reduce(idxf[:], cand[:], mybir.AxisListType.X,
                                            ALU.min)
                    idxi = route_p.tile([1, 1], mybir.dt.int32, tag=f"idxi{k}")
                    nc.vector.tensor_copy(idxi[:], idxf[:])
                    idx_i.append(idxi)

                # ACT builds the wd/wu gather descriptors, GpSimd (Pool) the
                # bias ones: each engine has its own 49-register file and one
                # file can't hold all the dynamic address expressions
                ivals = [
                    nc.values_load(
                        idx_i[k][0:1, 0:1],
                        engines=[mybir.EngineType.Activation,
                                 mybir.EngineType.Pool],
                        min_val=0, max_val=E - 1, skip_runtime_bounds_check=True,
                    )
                    for k in range(2)
                ]

                # gather the two experts' params (dynamic DMA); slot-k data
                # lives at slot-k offsets in the host-doubled tables, so each
                # gather is one simple strided DMA
                wd_mm = wg_p.tile([128, NDC, 128], BF16, tag="wdg",
                                  name=f"wd_{b}")
                for k in range(2):
                    nc.scalar.dma_start(
                        wd_mm[:, :, 64 * k:64 * (k + 1)],
                        wdt2[bass.ds(ivals[k], 1), :, :, 64 * k:64 * (k + 1)]
                        .rearrange("o p dc c -> (o p) dc c"),
                    )
                wu_g = wg_p.tile([128, D], BF16, tag="wug", name=f"wug_{b}")
                for k in range(2):
                    nc.scalar.dma_start(
                        wu_g[64 * k:64 * (k + 1), :],
                        wut2[bass.ds(ivals[k], 1), 64 * k:64 * (k + 1), :]
                        .rearrange("o c d -> (o c) d"),
                    )
                bb = []
                for k in range(2):
                    bbk = route_p.tile([128, 1 + NDC], F32, tag=f"bb{k}",
                                       name=f"bb{k}_{b}")
                    nc.gpsimd.dma_start(
                        bbk[:],
                        bcomb[bass.ds(ivals[k], 1), :, :]
                        .rearrange("o p f -> (o p) f"),
                    )
                    bb.append(bbk)

                # bounce wts through DRAM so 0-stride partition-broadcast
                # reads are legal (SBUF sources need nonzero partition step)
                nc.gpsimd.dma_start(wts_dram[b][:], wts[:])
                wfull = route_p.tile([128, 2], F32, tag="wfull", name=f"wf_{b}")
                for k in range(2):
                    nc.gpsimd.dma_start(
                        wfull[:, k:k + 1],
                        wts_dram[b][0:1, k:k + 1].to_broadcast((128, 1)),
                    )
                # per-slot weight column for the wu scale (rows 0-63 get w0,
                # 64-127 get w1)
                wcol = route_p.tile([128, 1], F32, tag="wcol", name=f"wcol_{b}")
                for k in range(2):
                    nc.vector.tensor_copy(wcol[64 * k:64 * (k + 1), :],
                                          wfull[64 * k:64 * (k + 1), k:k + 1])

                # gelu bias column: slot-k rows of bb[k] col 0
                bd_col = route_p.tile([128, 1], F32, tag="bdcol", name=f"bd_{b}")
                for k in range(2):
                    nc.vector.tensor_copy(bd_col[64 * k:64 * (k + 1), :],
                                          bb[k][64 * k:64 * (k + 1), 0:1])
                # combined routed up-bias, column form per dh chunk
                t0 = route_p.tile([128, NDC], F32, tag="bt0")
                nc.vector.tensor_scalar(t0[:], bb[0][:, 1:1 + NDC],
                                        wfull[:, 0:1], None, ALU.mult)
                t1b = route_p.tile([128, NDC], F32, tag="bt1")
                nc.vector.tensor_scalar(t1b[:], bb[1][:, 1:1 + NDC],
                                        wfull[:, 1:2], None, ALU.mult)
                bias_c = route_p.tile([128, NDC], F32, tag="biasc",
                                      name=f"bc_{b}")
                nc.vector.tensor_add(bias_c[:], t0[:], t1b[:])

                # fold routing weight into the up weights (bf16 again)
                wu_s = wg_p.tile([128, D], BF16, tag="wus", name=f"wus_{b}")
                nc.vector.tensor_scalar(wu_s[:], wu_g[:], wcol[:], None, ALU.mult)

                st.update(wd=wd_mm, wu=wu_s, bd=bd_col, bc=bias_c)

            def phase2(b, pool_b=None, logits_b=None):
                """Down+gelu+up+copies+stores for sample b. Interleaves the
                pooling reduces of sample `pool_b` and the logit matmul of
                sample `logits_b` into the up loop so every engine stream
                stays in dataflow order."""
                st = state[b]
                xt, wd_mm, wu_s = st["xt"], st["wd"], st["wu"]
                bd_col, bias_c = st["bd"], st["bc"]

                ht = ht_p.tile([128, S], BF16, tag="ht", name=f"ht_{b}")
                for sh in range(2):
                    h_ps = hps_p.tile([128, 1024], F32, tag="hps",
                                      name=f"hps_{b}_{sh}")
                    for dc in range(NDC):
                        for q in range(2):
                            s0 = sh * 1024 + q * 512
                            nc.tensor.matmul(
                                h_ps[:, q * 512:(q + 1) * 512],
                                wd_mm[:, dc, :], xt[dc][:, s0:s0 + 512],
                                start=(dc == 0), stop=(dc == NDC - 1),
                            )
                    nc.scalar.activation(
                        ht[:, sh * 1024:(sh + 1) * 1024], h_ps[:],
                        AF.Gelu, bias=bd_col[:],
                    )

                for dh in range(NDC):
                    if pool_b is not None:
                        pool_chunk(pool_b, dh)
                    if logits_b is not None and dh == 4:
                        logits(logits_b)
                    o_sb = osb_p.tile([128, S], BF16, tag="osb",
                                      name=f"osb_{b}_{dh}")
                    for sh in range(2):
                        o_ps = ops_p.tile([128, 1024], F32, tag="ops",
                                          name=f"ops_{b}_{dh}_{sh}")
                        for q in range(2):
                            s0 = sh * 1024 + q * 512
                            nc.tensor.matmul(
                                o_ps[:, q * 512:(q + 1) * 512],
                                wu_s[:, dh * 128:(dh + 1) * 128],
                                ht[:, s0:s0 + 512],
                                start=True, stop=True,
                            )
                        dst = o_sb[:, sh * 1024:(sh + 1) * 1024]
                        bcol = bias_c[:, dh:dh + 1]
                        if COPY_ROT[dh * 2 + sh] == "s":
                            nc.scalar.activation(dst, o_ps[:], AF.Identity,
                                                 bias=bcol)
                        else:
                            nc.vector.tensor_scalar_add(dst, o_ps[:], bcol)
                    # stores on gpsimd's SWDGE ring keep the SP ring free
                    nc.gpsimd.dma_start(out_t[b, dh * 128:(dh + 1) * 128, :],
                                        o_sb[:])

            # ---- schedule ----
            for b in range(BPC):
                loads(b)
            for dc in range(NDC):
                pool_chunk(0, dc)
            logits(0)
            route(0)
            for dc in range(NDC):
                pool_chunk(1, dc)
            phase2(0, pool_b=2, logits_b=1)
            route(1)
            phase2(1, pool_b=3, logits_b=2)
            route(2)
            phase2(2, logits_b=3)
            route(3)
            phase2(3)

    return nc


_NC_CACHE = {}


def _get_nc():
    if "nc" not in _NC_CACHE:
        nc = build_nc()
        _split_multiwait(nc)  # after build: walrus wants <=1 wait per inst
        _NC_CACHE["nc"] = nc
    return _NC_CACHE["nc"]


def make_in_maps(x, gate_w, down_w, down_b, up_w, up_b):
    import ml_dtypes

    bf16 = ml_dtypes.bfloat16
    wdt = down_w.transpose(0, 2, 1).reshape(E, NDC, 128, BN).transpose(0, 2, 1, 3)
    wdt2 = np.concatenate([wdt, wdt], axis=3).astype(bf16)      # [E,128,NDC,128]
    wut = up_w.transpose(0, 2, 1)                                # [E,BN,D]
    wut2 = np.concatenate([wut, wut], axis=1).astype(bf16)       # [E,128,D]
    bcomb = np.concatenate(
        [np.tile(down_b, (1, 2))[:, :, None],                    # [E,128,1]
         up_b.reshape(E, NDC, 128).transpose(0, 2, 1)],          # [E,128,NDC]
        axis=2,
    ).astype(np.float32)
    shared = {
        "gwt": (np.ascontiguousarray(gate_w.T) / np.float32(S)).astype(np.float32),
        "wdt2": np.ascontiguousarray(wdt2),
        "wut2": np.ascontiguousarray(wut2),
        "bcomb": np.ascontiguousarray(bcomb),
        "iota8": np.arange(E, dtype=np.float32).reshape(1, E),
    }
    in_maps = []
    for c in range(NCORES):
        m = dict(shared)
        m["xt"] = np.ascontiguousarray(
            x[c * BPC:(c + 1) * BPC].transpose(0, 2, 1).astype(bf16)
        )
        in_maps.append(m)
    return in_maps


def kernel(x, gate_w, down_w, down_b, up_w, up_b, _trace=False):
    from concourse.bass_utils import run_bass_kernel_spmd

    nc = _get_nc()
    in_maps = make_in_maps(x, gate_w, down_w, down_b, up_w, up_b)
    res = run_bass_kernel_spmd(nc, in_maps, list(range(NCORES)), trace=_trace)
    out = np.concatenate(
        [res.results[c]["out"].astype(np.float32).transpose(0, 2, 1)
         for c in range(NCORES)],
        axis=0,
    )
    if _trace:
        kernel.last_result = res
    return out


# revision 13
# speedup vs baseline: 1.0655x; 1.0655x over previous
"""MoE adapter layer kernel for Trainium2 (8 NeuronCores, data-parallel over B).

Reference computation (per sample b):
    pooled = x[b].mean(axis=0)                       # (D,)
    gate   = softmax(pooled @ gate_w.T)              # (E,)
    top2 values/indices, renormalized weights w0,w1
    h_k    = gelu(x[b] @ Wd[ik].T + bd[ik])          # (S, BN)
    out[b] = sum_k w_k * h_k @ Wu[ik].T + sum_k w_k * bu[ik]

Shapes: B=32, S=2048, D=1024, BN=64, E=8, K=2. Inputs fp32.

Strategy: shard B over the 8 cores (4 samples each); replicate the tiny
adapter/gate params. All matmul operands are cast to bf16 on the host
(PE runs 1 cycle/row in bf16 vs 4 in fp32, and HBM traffic halves); the
bf16 rounding of x perturbs the routing logits 2 orders of magnitude
less than the smallest top-2 decision margin, and the end-to-end error
stays ~5e-3 relative vs the 2e-2 gate.

Per core the layer is DMA-bound-ish (~36 MB of HBM traffic) with every
compute engine moderately loaded, so the schedule is built around two
rules: the SP DMA ring carries ONLY the x^T loads (any routing-gated
DMA there stalls the next samples' loads behind a semaphore), and no
engine's program order may make an early sample's work wait on a later
sample's data. Concretely: loads for all 4 samples issue up front
(all of x fits in SBUF in bf16); pooling is per-tile DVE/ACT/GpSimd
reduces that chase the loads, with samples 2-3's reduces interleaved
into the up-matmul loop of samples 0-1; the next sample's gate-logit
matmul is dropped into the middle of the current up loop (right when
its pooling completes); routing needs no softmax (top-2 on logits; the
renormalized pair weight is exactly sigmoid(l1-l2), one ACT op);
expert params are gathered with dynamically-indexed DMAs from
host-prearranged slot-doubled tables on the ACT/GpSimd rings; the
output is produced transposed (d on partitions) so the routed up-bias
is a per-partition scalar fused into the PSUM->SBUF bf16 down-convert,
which alternates ACT/DVE to keep pace with the PE; stores run on the
GpSimd ring. The host un-transposes and upcasts.
"""

import sys

sys.path.insert(0, "/opt/trn_rl_repo")

import numpy as np

import concourse.bass as bass
import concourse.mybir as mybir
import concourse.tile as tile

F32 = mybir.dt.float32
BF16 = mybir.dt.bfloat16
AF = mybir.ActivationFunctionType
ALU = mybir.AluOpType

B, S, D, BN, E = 32, 2048, 1024, 64, 8
NCORES = 8
BPC = B // NCORES  # samples per core
NDC = D // 128     # 8 d-chunks of 128

# engine per (dh, sh) output chunk copy: "s"=ACT, "v"=DVE (gpsimd cannot
# read PSUM). ACT leads so the first chunks drain while DVE finishes the
# next sample's routing.
COPY_ROT = ["s", "s", "v", "v", "s", "v", "s", "v",
            "s", "v", "s", "v", "s", "v", "s", "v"]
# pooling engine per dc chunk: DVE reduce has no fast modes, so ACT
# (activation accum) takes a share to keep DVE off the critical path
POOL_ENG = ["v", "v", "v", "v", "v", "v", "s", "s"]


def _split_multiwait(nc):
    """The pinned walrus encodes at most one sync-wait per instruction;
    hoist extra waits into standalone EventSemaphore instructions."""
    fixn = 0
    for f in nc.m.functions:
        for b in f.blocks:
            if not any(
                i.sync_info is not None
                and i.sync_info.on_wait is not None
                and len(i.sync_info.on_wait) > 1
                for i in b.instructions
            ):
                continue
            out = []
            for inst in b.instructions:
                si = inst.sync_info
                if si is not None and si.on_wait is not None and len(si.on_wait) > 1:
                    waits = list(si.on_wait)
                    for w in waits[:-1]:
                        ev = mybir.InstEventSemaphore(
                            name=f"I-mwfix-{fixn}", engine=inst.engine
                        )
                        ev.sync_info = mybir.SyncInfo(on_wait=[w], on_update=[])
                        out.append(ev)
                        fixn += 1
                    inst.sync_info = mybir.SyncInfo(
                        on_wait=[waits[-1]],
                        on_update=list(si.on_update) if si.on_update else [],
                    )
                out.append(inst)
            b.instructions = out
    return fixn


def build_nc():
    """Build the per-core Bass program (SPMD: same program, different x shard)."""
    nc = bass.Bass()

    # x arrives pre-transposed per sample: (BPC, D, S) bf16 so the down
    # matmul's moving operand (contraction over D -> D on partitions) DMAs
    # naturally.
    xt_in = nc.dram_tensor("xt", [BPC, D, S], BF16, kind="ExternalInput")
    gwt = nc.dram_tensor("gwt", [D, E], F32, kind="ExternalInput")  # gate_w.T/S
    # wdt2[e, p, dc, 64k+c] = down_w[e, c, dc*128+p] for both k slots
    wdt2 = nc.dram_tensor("wdt2", [E, 128, NDC, 128], BF16, kind="ExternalInput")
    # wut2[e, 64k+p, d] = up_w[e, d, p] for both k slots
    wut2 = nc.dram_tensor("wut2", [E, 128, D], BF16, kind="ExternalInput")
    # bcomb[e, p, 0] = down_b[e, p % 64]; bcomb[e, p, 1+dh] = up_b[e, dh*128+p]
    bcomb = nc.dram_tensor("bcomb", [E, 128, 1 + NDC], F32, kind="ExternalInput")
    iota8 = nc.dram_tensor("iota8", [1, E], F32, kind="ExternalInput")
    out_t = nc.dram_tensor("out", [BPC, D, S], BF16, kind="ExternalOutput")
    wts_dram = [nc.dram_tensor(f"wts_scratch_{b}", [1, 2], F32) for b in range(BPC)]

    with tile.TileContext(nc) as tc:
        with (
            tc.tile_pool(name="singles", bufs=1) as singles,
            tc.tile_pool(name="xt", bufs=4 * NDC) as xt_p,
            tc.tile_pool(name="ht", bufs=2) as ht_p,
            tc.tile_pool(name="wg", bufs=2) as wg_p,
            tc.tile_pool(name="osb", bufs=3) as osb_p,
            tc.tile_pool(name="route", bufs=2) as route_p,
            tc.tile_pool(name="hps", bufs=2, space="PSUM") as hps_p,
            tc.tile_pool(name="ops", bufs=2, space="PSUM") as ops_p,
        ):
            gwt_sb = singles.tile([128, NDC, E], F32, tag="gwt")
            nc.sync.dma_start(gwt_sb[:], gwt.rearrange("(dc p) e -> p dc e", p=128))
            iota_sb = singles.tile([1, E], F32, tag="iota")
            nc.sync.dma_start(iota_sb[:], iota8[:])

            state = {b: {} for b in range(BPC)}

            def loads(b):
                # SP ring carries nothing but these
                xt = []
                for dc in range(NDC):
                    xt_sb = xt_p.tile([128, S], BF16, tag="xt", name=f"xt_{b}_{dc}")
                    nc.sync.dma_start(xt_sb[:], xt_in[b, dc * 128:(dc + 1) * 128, :])
                    xt.append(xt_sb)
                state[b]["xt"] = xt
                state[b]["pooled"] = route_p.tile([128, NDC], F32, tag="pooled",
                                                  name=f"pooled_{b}")

            def pool_chunk(b, dc):
                pooled, xt_sb = state[b]["pooled"], state[b]["xt"][dc]
                dst = pooled[:, dc:dc + 1]
                if POOL_ENG[dc] == "v":
                    nc.vector.tensor_reduce(dst, xt_sb[:], mybir.AxisListType.X,
                                            ALU.add)
                else:
                    scr = route_p.tile([128, S], BF16, tag="scr",
                                       name=f"scr_{b}_{dc}")
                    nc.scalar.activation(scr[:], xt_sb[:], AF.Copy, accum_out=dst)

            def logits(b):
                # PE: 8 tiny rank-128 matmuls into an ops-pool corner
                pooled = state[b]["pooled"]
                lps = ops_p.tile([128, 1024], F32, tag="ops", name=f"lps_{b}")
                for dc in range(NDC):
                    nc.tensor.matmul(
                        lps[0:1, 0:E], pooled[:, dc:dc + 1], gwt_sb[:, dc, :],
                        start=(dc == 0), stop=(dc == NDC - 1),
                    )
                state[b]["lps"] = lps

            def route(b):
                st = state[b]
                logits_r = route_p.tile([1, E], F32, tag="logits", name=f"lg_{b}")
                nc.vector.tensor_copy(logits_r[:], st["lps"][0:1, 0:E])

                # top-2 of logits; pair weight = sigmoid(l1-l2) (exactly the
                # renormalized softmax pair, sans the reference's 1e-8 eps)
                m8 = route_p.tile([1, E], F32, tag="m8")
                nc.vector.max(m8[:], logits_r[:])
                ds2 = route_p.tile([1, 2], F32, tag="ds2")
                nc.vector.tensor_sub(ds2[:, 0:1], m8[:, 0:1], m8[:, 1:2])
                nc.vector.tensor_scalar_mul(ds2[:, 1:2], ds2[:, 0:1], -1.0)
                wts = route_p.tile([1, 2], F32, tag="wts")
                nc.scalar.activation(wts[:], ds2[:], AF.Sigmoid)

                idx_i = []
                for k in range(2):
                    eq = route_p.tile([1, E], F32, tag=f"eq{k}")
                    nc.vector.tensor_scalar(eq[:], logits_r[:], m8[:, k:k + 1],
                                            None, ALU.is_equal)
                    # cand = iota*eq + 99*(1-eq): first matching index wins min
                    t1 = route_p.tile([1, E], F32, tag=f"t1_{k}")
                    nc.vector.tensor_mul(t1[:], iota_sb[:], eq[:])
                    t2 = route_p.tile([1, E], F32, tag=f"t2_{k}")
                    nc.vector.tensor_scalar(t2[:], eq[:], -99.0, 99.0, ALU.mult,
                                            ALU.add)
                    cand = route_p.tile([1, E], F32, tag=f"cand{k}")
                    nc.vector.tensor_add(cand[:], t1[:], t2[:])
                    idxf = route_p.tile([1, 1], F32, tag=f"idxf{k}")
                    nc.vector.tensor_reduce(idxf[:], cand[:], mybir.AxisListType.X,
                                            ALU.min)
                    idxi = route_p.tile([1, 1], mybir.dt.int32, tag=f"idxi{k}")
                    nc.vector.tensor_copy(idxi[:], idxf[:])
                    idx_i.append(idxi)

                # ACT builds the wd/wu gather descriptors, GpSimd (Pool) the
                # bias ones: each engine has its own 49-register file and one
                # file can't hold all the dynamic address expressions
                ivals = [
                    nc.values_load(
                        idx_i[k][0:1, 0:1],
                        engines=[mybir.EngineType.Activation,
                                 mybir.EngineType.Pool],
                        min_val=0, max_val=E - 1, skip_runtime_bounds_check=True,
                    )
                    for k in range(2)
                ]

                # gather the two experts' params (dynamic DMA); slot-k data
                # lives at slot-k offsets in the host-doubled tables, so each
                # gather is one simple strided DMA
                wd_mm = wg_p.tile([128, NDC, 128], BF16, tag="wdg",
                                  name=f"wd_{b}")
                for k in range(2):
                    nc.scalar.dma_start(
                        wd_mm[:, :, 64 * k:64 * (k + 1)],
                        wdt2[bass.ds(ivals[k], 1), :, :, 64 * k:64 * (k + 1)]
                        .rearrange("o p dc c -> (o p) dc c"),
                    )
                wu_g = wg_p.tile([128, D], BF16, tag="wug", name=f"wug_{b}")
                for k in range(2):
                    nc.scalar.dma_start(
                        wu_g[64 * k:64 * (k + 1), :],
                        wut2[bass.ds(ivals[k], 1), 64 * k:64 * (k + 1), :]
                        .rearrange("o c d -> (o c) d"),
                    )
                bb = []
                for k in range(2):
                    bbk = route_p.tile([128, 1 + NDC], F32, tag=f"bb{k}",
                                       name=f"bb{k}_{b}")
                    nc.gpsimd.dma_start(
                        bbk[:],
                        bcomb[bass.ds(ivals[k], 1), :, :]
                        .rearrange("o p f -> (o p) f"),
                    )
                    bb.append(bbk)

                # bounce wts through DRAM so 0-stride partition-broadcast
                # reads are legal (SBUF sources need nonzero partition step)
                nc.gpsimd.dma_start(wts_dram[b][:], wts[:])
                wfull = route_p.tile([128, 2], F32, tag="wfull", name=f"wf_{b}")
                for k in range(2):
                    nc.gpsimd.dma_start(
                        wfull[:, k:k + 1],
                        wts_dram[b][0:1, k:k + 1].to_broadcast((128, 1)),
                    )
                # per-slot weight column for the wu scale (rows 0-63 get w0,
                # 64-127 get w1)
                wcol = route_p.tile([128, 1], F32, tag="wcol", name=f"wcol_{b}")
                for k in range(2):
                    nc.vector.tensor_copy(wcol[64 * k:64 * (k + 1), :],
                                          wfull[64 * k:64 * (k + 1), k:k + 1])

                # gelu bias column: slot-k rows of bb[k] col 0
                bd_col = route_p.tile([128, 1], F32, tag="bdcol", name=f"bd_{b}")
                for k in range(2):
                    nc.vector.tensor_copy(bd_col[64 * k:64 * (k + 1), :],
                                          bb[k][64 * k:64 * (k + 1), 0:1])
                # combined routed up-bias, column form per dh chunk
                t0 = route_p.tile([128, NDC], F32, tag="bt0")
                nc.vector.tensor_scalar(t0[:], bb[0][:, 1:1 + NDC],
                                        wfull[:, 0:1], None, ALU.mult)
                t1b = route_p.tile([128, NDC], F32, tag="bt1")
                nc.vector.tensor_scalar(t1b[:], bb[1][:, 1:1 + NDC],
                                        wfull[:, 1:2], None, ALU.mult)
                bias_c = route_p.tile([128, NDC], F32, tag="biasc",
                                      name=f"bc_{b}")
                nc.vector.tensor_add(bias_c[:], t0[:], t1b[:])

                # fold routing weight into the up weights (bf16 again)
                wu_s = wg_p.tile([128, D], BF16, tag="wus", name=f"wus_{b}")
                nc.vector.tensor_scalar(wu_s[:], wu_g[:], wcol[:], None, ALU.mult)

                st.update(wd=wd_mm, wu=wu_s, bd=bd_col, bc=bias_c)

            def phase2(b, pool_b=None, logits_b=None):
                """Down+gelu+up+copies+stores for sample b. Interleaves the
                pooling reduces of sample `pool_b` and the LOGITS+ROUTING of
                sample `logits_b` into the up loop (logits at dh 4, routing
                right after) so routing_{b+1} resolves mid-up instead of
                queueing behind all of sample b's copies on DVE."""
                st = state[b]
                xt, wd_mm, wu_s = st["xt"], st["wd"], st["wu"]
                bd_col, bias_c = st["bd"], st["bc"]

                ht = ht_p.tile([128, S], BF16, tag="ht", name=f"ht_{b}")
                for sh in range(2):
                    h_ps = hps_p.tile([128, 1024], F32, tag="hps",
                                      name=f"hps_{b}_{sh}")
                    for dc in range(NDC):
                        for q in range(2):
                            s0 = sh * 1024 + q * 512
                            nc.tensor.matmul(
                                h_ps[:, q * 512:(q + 1) * 512],
                                wd_mm[:, dc, :], xt[dc][:, s0:s0 + 512],
                                start=(dc == 0), stop=(dc == NDC - 1),
                            )
                    nc.scalar.activation(
                        ht[:, sh * 1024:(sh + 1) * 1024], h_ps[:],
                        AF.Gelu, bias=bd_col[:],
                    )

                for dh in range(NDC):
                    if pool_b is not None:
                        pool_chunk(pool_b, dh)
                    if logits_b is not None and dh == 4:
                        logits(logits_b)
                    if logits_b is not None and dh == 5:
                        route(logits_b)
                    o_sb = osb_p.tile([128, S], BF16, tag="osb",
                                      name=f"osb_{b}_{dh}")
                    for sh in range(2):
                        o_ps = ops_p.tile([128, 1024], F32, tag="ops",
                                          name=f"ops_{b}_{dh}_{sh}")
                        for q in range(2):
                            s0 = sh * 1024 + q * 512
                            nc.tensor.matmul(
                                o_ps[:, q * 512:(q + 1) * 512],
                                wu_s[:, dh * 128:(dh + 1) * 128],
                                ht[:, s0:s0 + 512],
                                start=True, stop=True,
                            )
                        dst = o_sb[:, sh * 1024:(sh + 1) * 1024]
                        bcol = bias_c[:, dh:dh + 1]
                        if COPY_ROT[dh * 2 + sh] == "s":
                            nc.scalar.activation(dst, o_ps[:], AF.Identity,
                                                 bias=bcol)
                        else:
                            nc.vector.tensor_scalar_add(dst, o_ps[:], bcol)
                    # stores on gpsimd's SWDGE ring keep the SP ring free
                    nc.gpsimd.dma_start(out_t[b, dh * 128:(dh + 1) * 128, :],
                                        o_sb[:])

            # ---- schedule ----
            for b in range(BPC):
                loads(b)
            for dc in range(NDC):
                pool_chunk(0, dc)
            logits(0)
            route(0)
            for dc in range(NDC):
                pool_chunk(1, dc)
            phase2(0, pool_b=2, logits_b=1)
            phase2(1, pool_b=3, logits_b=2)
            phase2(2, logits_b=3)
            phase2(3)

    return nc


_NC_CACHE = {}


def _get_nc():
    if "nc" not in _NC_CACHE:
        nc = build_nc()
        _split_multiwait(nc)  # after build: walrus wants <=1 wait per inst
        _NC_CACHE["nc"] = nc
    return _NC_CACHE["nc"]


def make_in_maps(x, gate_w, down_w, down_b, up_w, up_b):
    import ml_dtypes

    bf16 = ml_dtypes.bfloat16
    wdt = down_w.transpose(0, 2, 1).reshape(E, NDC, 128, BN).transpose(0, 2, 1, 3)
    wdt2 = np.concatenate([wdt, wdt], axis=3).astype(bf16)      # [E,128,NDC,128]
    wut = up_w.transpose(0, 2, 1)                                # [E,BN,D]
    wut2 = np.concatenate([wut, wut], axis=1).astype(bf16)       # [E,128,D]
    bcomb = np.concatenate(
        [np.tile(down_b, (1, 2))[:, :, None],                    # [E,128,1]
         up_b.reshape(E, NDC, 128).transpose(0, 2, 1)],          # [E,128,NDC]
        axis=2,
    ).astype(np.float32)
    shared = {
        "gwt": (np.ascontiguousarray(gate_w.T) / np.float32(S)).astype(np.float32),
        "wdt2": np.ascontiguousarray(wdt2),
        "wut2": np.ascontiguousarray(wut2),
        "bcomb": np.ascontiguousarray(bcomb),
        "iota8": np.arange(E, dtype=np.float32).reshape(1, E),
    }
    in_maps = []
    for c in range(NCORES):
        m = dict(shared)
        m["xt"] = np.ascontiguousarray(
            x[c * BPC:(c + 1) * BPC].transpose(0, 2, 1).astype(bf16)
        )
        in_maps.append(m)
    return in_maps


def kernel(x, gate_w, down_w, down_b, up_w, up_b, _trace=False):
    from concourse.bass_utils import run_bass_kernel_spmd

    nc = _get_nc()
    in_maps = make_in_maps(x, gate_w, down_w, down_b, up_w, up_b)
    res = run_bass_kernel_spmd(nc, in_maps, list(range(NCORES)), trace=_trace)
    out = np.concatenate(
        [res.results[c]["out"].astype(np.float32).transpose(0, 2, 1)
         for c in range(NCORES)],
        axis=0,
    )
    if _trace:
        kernel.last_result = res
    return out
